# revision 7
# baseline (speedup 1.0000x reference)
"""CenterLoss kernel for Trainium2 (8 NeuronCores, data-parallel over batch).

loss = mean_i( clip( ||x_i - centers[labels[i]]||^2, 1e-12, 1e12 ) )

Instead of materializing the [B, C] distance matrix (as the reference does),
we gather the labeled center row per sample and compute the squared distance
directly: O(B*D) work instead of O(B*C*D).

Sharding: x/labels split into 8 batch shards of 1024 rows; centers replicated.
Each core emits its shard's clipped per-sample distances; the host sums the
8 partials and divides by the global batch (the sanctioned all-reduce).

v3 changes vs the 29.7us v1 baseline (trace-driven):
  - SWDGE descriptor-gen costs ~994 ns fixed per instruction + 0.34 ns/desc;
    v1 paid the fixed cost 8x (8 indirect gathers of 128 rows, ~8.8 us
    serialized on GpSimd). The multi-offset form of indirect_dma_start is
    broken in the Q7 ucode (only one offset per partition honored), so the
    batched gather uses the MoE dma_gather ucode instead: 2 calls of 512
    rows each (~2.3 us total desc-gen), int16 indices wrapped 16-wide.
  - x and centers ship as bf16 (host-side cast): halves HBM traffic
    (4 MB -> 2 MB per core) and doubles DVE element rate. Squared distances
    accumulate in fp32 (ScalarE activation accumulator / DVE
    tensor_tensor_reduce), keeping the loss at ~1e-3 relative error vs the
    2e-2 gate.
  - idx loads via the scalar engine's HWDGE ring (clears the framework
    preamble ~1 us before sync); x via sync's ring, host-pre-permuted to
    the gather's (partition, chunk) layout so the DMA is contiguous.
  - per-chunk squared-row-sums split ScalarE (fused Square+accum) / DVE
    (fused tensor_tensor_reduce); DVE also does the 8 bf16 subtracts.

Per-core layout (B_loc=1024, P=128 partitions, M=8 chunks): dma_gather
writes gathered row j of a call to (partition j%128, chunk j//128), so
sample s sits at (p, c) = (s%128, s//128); x is host-permuted to match.
"""

import sys

import numpy as np

if "/opt/trn_rl_repo" not in sys.path:
    sys.path.insert(0, "/opt/trn_rl_repo")

_B, _D, _C = 8192, 512, 8000
_N_CORES = 8
_B_LOC = _B // _N_CORES  # 1024 rows per core
_P = 128
_M = _B_LOC // _P  # 8 chunks of 128 rows
_N_GATHER = 2  # dma_gather calls per core
_ROWS = _B_LOC // _N_GATHER  # rows per gather call (512)
_CLAMP_MIN, _CLAMP_MAX = 1e-12, 1e12

# engine assignment for the per-chunk squared row-sum
_SCALAR_SQ = (0, 1, 4, 5)  # ScalarE: fused Square + fp32 accumulator
# remaining chunks: DVE fused tensor_tensor_reduce (mult, add)

_cache: dict = {}


def _build():
    import concourse.tile as tile
    from concourse import bacc, mybir

    nc = bacc.Bacc(
        "TRN2",
        debug=False,
        enable_asserts=False,
        target_bir_lowering=False,
        num_devices=_N_CORES,
    )
    # x ships host-permuted: x_perm[p, c*D:(c+1)*D] = x_shard[c*128 + p]
    x_d = nc.dram_tensor("x_perm", [_P, _M * _D], mybir.dt.bfloat16, kind="ExternalInput")
    # labels ship as int16, 16-partition-wrapped per dma_gather's layout
    lab_d = nc.dram_tensor(
        "labels_packed", [_P, _N_GATHER * (_ROWS // 16)], mybir.dt.int16,
        kind="ExternalInput",
    )
    cen_d = nc.dram_tensor("centers", [_C, _D], mybir.dt.bfloat16, kind="ExternalInput")
    out_d = nc.dram_tensor("out", [_P, _M], mybir.dt.float32, kind="ExternalOutput")

    with tile.TileContext(nc) as tc:
        with (
            tc.tile_pool(name="big", bufs=1) as big,
            tc.tile_pool(name="work", bufs=4) as work,
            tc.tile_pool(name="misc", bufs=1) as misc,
        ):
            wcols = _ROWS // 16  # idx columns per gather call (32)
            idx = misc.tile([_P, _N_GATHER * wcols], mybir.dt.int16)
            # idx gates gather descriptor-gen; the scalar engine's HWDGE ring
            # clears the framework preamble earliest.
            nc.scalar.dma_start(out=idx[:], in_=lab_d.ap())

            xsb = big.tile([_P, _M * _D], mybir.dt.bfloat16)
            nc.sync.dma_start(out=xsb[:], in_=x_d.ap())

            dist = misc.tile([_P, _M], mybir.dt.float32)

            g = big.tile([_P, _M * _D], mybir.dt.bfloat16)
            g3 = g[:].rearrange("p (m d) -> p m d", d=_D)
            mh = _M // _N_GATHER  # chunks per gather call
            for h in range(_N_GATHER):
                nc.gpsimd.dma_gather(
                    out_ap=g3[:, h * mh : (h + 1) * mh, :],
                    in_ap=cen_d.ap(),
                    idxs_ap=idx[:, h * wcols : (h + 1) * wcols],
                    num_idxs=_ROWS,
                    num_idxs_reg=_ROWS,
                    elem_size=_D,
                )
                for m in range(h * mh, (h + 1) * mh):
                    diff = work.tile([_P, _D], mybir.dt.bfloat16, tag="diff")
                    nc.vector.tensor_tensor(
                        out=diff[:],
                        in0=xsb[:, m * _D : (m + 1) * _D],
                        in1=g[:, m * _D : (m + 1) * _D],
                        op=mybir.AluOpType.subtract,
                    )
                    if m in _SCALAR_SQ:
                        sq = work.tile([_P, _D], mybir.dt.bfloat16, tag="sq")
                        nc.scalar.activation(
                            out=sq[:],
                            in_=diff[:],
                            func=mybir.ActivationFunctionType.Square,
                            accum_out=dist[:, m : m + 1],
                        )
                    else:
                        # (tensor_tensor_reduce is a custom DVE ISA op that
                        # crashes this runtime; use mult + reduce instead)
                        sq = work.tile([_P, _D], mybir.dt.bfloat16, tag="sqv")
                        nc.vector.tensor_tensor(
                            out=sq[:], in0=diff[:], in1=diff[:],
                            op=mybir.AluOpType.mult,
                        )
                        nc.vector.tensor_reduce(
                            out=dist[:, m : m + 1],
                            in_=sq[:],
                            axis=mybir.AxisListType.X,
                            op=mybir.AluOpType.add,
                        )

            # clip both bounds in one DVE op: out = min(max(dist, lo), hi).
            nc.vector.tensor_scalar(
                out=dist[:],
                in0=dist[:],
                scalar1=_CLAMP_MIN,
                scalar2=_CLAMP_MAX,
                op0=mybir.AluOpType.max,
                op1=mybir.AluOpType.min,
            )

            # Ship the clipped per-sample distances (4 KB); the host folds
            # them into the per-shard partial sums.
            nc.sync.dma_start(out=out_d.ap()[:, :], in_=dist[:])
    nc.compile()
    return nc


def _pack_labels(labels_shard: np.ndarray) -> np.ndarray:
    """int16 [128, 64]: per 512-row call h, index j at (j%16, h*32 + j//16),
    replicated across the 8 16-partition groups (dma_gather's idx layout)."""
    lab16 = labels_shard.astype(np.int16)
    cols = []
    for h in range(_N_GATHER):
        w = lab16[h * _ROWS : (h + 1) * _ROWS].reshape(_ROWS // 16, 16).T  # [16, 32]
        cols.append(np.tile(w, (_P // 16, 1)))  # [128, 32]
    return np.ascontiguousarray(np.concatenate(cols, axis=1))


def _run(x, labels, centers, trace=False, **hw_kwargs):
    import ml_dtypes
    from concourse import bass_utils

    if "nc" not in _cache:
        _cache["nc"] = _build()
    nc = _cache["nc"]

    x = np.asarray(x)
    labels = np.ascontiguousarray(np.asarray(labels).astype(np.int64))
    centers = np.asarray(centers)
    assert x.shape == (_B, _D) and labels.shape == (_B,) and centers.shape == (_C, _D)
    assert labels.min() >= 0 and labels.max() < _C

    x_bf = x.astype(ml_dtypes.bfloat16)
    cen_bf = np.ascontiguousarray(centers.astype(ml_dtypes.bfloat16))

    in_maps = []
    for c in range(_N_CORES):
        sl = slice(c * _B_LOC, (c + 1) * _B_LOC)
        # x_perm[p, c*D:(c+1)*D] = x_shard[c*128 + p]
        x_perm = np.ascontiguousarray(
            x_bf[sl].reshape(_M, _P, _D).transpose(1, 0, 2).reshape(_P, _M * _D)
        )
        in_maps.append(
            {
                "x_perm": x_perm,
                "labels_packed": _pack_labels(labels[sl]),
                "centers": cen_bf,
            }
        )

    r = bass_utils.run_bass_kernel_spmd(
        nc, in_maps, core_ids=list(range(_N_CORES)), trace=trace, **hw_kwargs
    )
    total = sum(res["out"].astype(np.float64).sum() for res in r.results)
    return np.array(total / _B, dtype=np.float32), r


def kernel(x, labels, centers):
    out, _ = _run(x, labels, centers, trace=False)
    return out


# revision 8
# speedup vs baseline: 1.2592x; 1.2592x over previous
"""CenterLoss kernel for Trainium2 (8 NeuronCores, data-parallel over batch).

loss = mean_i( clip( ||x_i - centers[labels[i]]||^2, 1e-12, 1e12 ) )

Instead of materializing the [B, C] distance matrix (as the reference does),
we gather the labeled center row per sample with indirect DMA and compute the
squared distance directly: O(B*D) work instead of O(B*C*D).

Sharding: x/labels split into 8 batch shards of 1024 rows; centers replicated.
Each core emits its shard's clipped per-sample distances; the host sums the
8 partials and divides by the global batch (the sanctioned all-reduce).

v4 design notes (trace-driven; this runtime's measured costs):
  - SWDGE descriptor-gen runs at ~9 ns/descriptor regardless of ucode
    (1024 gather descriptors ~ 9.2 us of Q7 time, serialized). The MoE
    dma_gather ucode batches calls but needs a library reload that costs
    ~12.6 us on this runtime, so mainline indirect_dma_start (no reload)
    with 8 gathers of 128 rows wins.
  - x and centers ship as bf16 (host-side cast): halves HBM traffic and
    speeds DVE. The Q7 indirect-DMA ucode miscomputes addresses for 2-byte
    dtypes, so centers are DECLARED fp32 [8000, 256] (bf16 pairs) and the
    gathered tile is bitcast back to bf16 for compute; address math then
    matches the working fp32 path exactly (1 KB rows).
  - idx loads via gpsimd SWDGE: the Q7 engine clears the framework preamble
    earliest (~6.1 us) and its queue is otherwise idle until the gathers;
    the scalar (Activation) HWDGE ring has ~4 us launch latency - avoid.
  - x loads in 2 halves on sync's HWDGE ring so the first 4 chunks can
    start compute ~2 us before the full 1 MB lands.
  - squared-row-sums: ScalarE takes 6 chunks (fused Square + fp32
    accumulator), DVE takes 2 (mult + reduce) plus all 8 bf16 subtracts.
  - loss error vs fp32 reference ~1e-3 << the 2e-2 gate (bf16 inputs,
    fp32 accumulation).

Per-core layout (B_loc=1024, P=128 partitions, M=8 chunks): sample s sits at
(partition p, chunk c) = (s%128, s//128); x is host-permuted to match, and
idx[p, c] = labels[c*128 + p].
"""

import sys

import numpy as np

if "/opt/trn_rl_repo" not in sys.path:
    sys.path.insert(0, "/opt/trn_rl_repo")

_B, _D, _C = 8192, 512, 8000
_N_CORES = 8
_B_LOC = _B // _N_CORES  # 1024 rows per core
_P = 128
_M = _B_LOC // _P  # 8 chunks of 128 rows
_DH = _D // 2  # fp32-typed width of a bf16 row (256)
_CLAMP_MIN, _CLAMP_MAX = 1e-12, 1e12

# engine assignment for the per-chunk squared row-sum
_DVE_SQ = (3, 7)  # DVE: mult + reduce; ScalarE takes the other 6

_cache: dict = {}


def _build():
    import concourse.bass as bass
    import concourse.tile as tile
    from concourse import bacc, mybir

    nc = bacc.Bacc(
        "TRN2",
        debug=False,
        enable_asserts=False,
        target_bir_lowering=False,
        num_devices=_N_CORES,
    )
    # x ships host-permuted: x_perm[p, c*D:(c+1)*D] = x_shard[c*128 + p] (bf16)
    x_d = nc.dram_tensor("x_perm", [_P, _M * _D], mybir.dt.bfloat16, kind="ExternalInput")
    # labels host-packed: idx[p, c] = labels[c*128 + p]
    lab_d = nc.dram_tensor("labels_packed", [_P, _M], mybir.dt.int32, kind="ExternalInput")
    # centers are bf16 pairs DECLARED fp32 so the gather's address math uses
    # the proven 4-byte path (1 KB per row either way).
    cen_d = nc.dram_tensor("centers_f32v", [_C, _DH], mybir.dt.float32, kind="ExternalInput")
    out_d = nc.dram_tensor("out", [_P, _M], mybir.dt.float32, kind="ExternalOutput")

    with tile.TileContext(nc) as tc:
        with (
            tc.tile_pool(name="big", bufs=1) as big,
            tc.tile_pool(name="work", bufs=4) as work,
            tc.tile_pool(name="misc", bufs=1) as misc,
        ):
            idx = misc.tile([_P, _M], mybir.dt.int32)
            # idx gates all gather descriptor-gen: Q7 SWDGE issues it the
            # moment gpsimd clears the preamble, ~1 us before sync could.
            nc.gpsimd.dma_start(out=idx[:], in_=lab_d.ap())

            # x in 2 halves so early chunks unblock before the full MB lands
            xsb = big.tile([_P, _M * _D], mybir.dt.bfloat16)
            half_el = (_M // 2) * _D
            nc.sync.dma_start(
                out=xsb[:, :half_el], in_=x_d.ap()[:, :half_el]
            )
            nc.sync.dma_start(
                out=xsb[:, half_el:], in_=x_d.ap()[:, half_el:]
            )

            dist = misc.tile([_P, _M], mybir.dt.float32)

            g = big.tile([_P, _M * _DH], mybir.dt.float32)
            g3 = g[:].rearrange("p (m d) -> p m d", d=_DH)
            gb = g[:].bitcast(mybir.dt.bfloat16)  # [128, M*512] bf16 view
            for m in range(_M):
                nc.gpsimd.indirect_dma_start(
                    out=g3[:, m, :],
                    out_offset=None,
                    in_=cen_d.ap(),
                    in_offset=bass.IndirectOffsetOnAxis(
                        ap=idx[:, m : m + 1], axis=0
                    ),
                )
                diff = work.tile([_P, _D], mybir.dt.bfloat16, tag="diff")
                nc.vector.tensor_tensor(
                    out=diff[:],
                    in0=xsb[:, m * _D : (m + 1) * _D],
                    in1=gb[:, m * _D : (m + 1) * _D],
                    op=mybir.AluOpType.subtract,
                )
                if m not in _DVE_SQ:
                    sq = work.tile([_P, _D], mybir.dt.bfloat16, tag="sq")
                    nc.scalar.activation(
                        out=sq[:],
                        in_=diff[:],
                        func=mybir.ActivationFunctionType.Square,
                        accum_out=dist[:, m : m + 1],
                    )
                else:
                    sq = work.tile([_P, _D], mybir.dt.bfloat16, tag="sqv")
                    nc.vector.tensor_tensor(
                        out=sq[:], in0=diff[:], in1=diff[:],
                        op=mybir.AluOpType.mult,
                    )
                    nc.vector.tensor_reduce(
                        out=dist[:, m : m + 1],
                        in_=sq[:],
                        axis=mybir.AxisListType.X,
                        op=mybir.AluOpType.add,
                    )

            # clip both bounds in one DVE op: out = min(max(dist, lo), hi).
            nc.vector.tensor_scalar(
                out=dist[:],
                in0=dist[:],
                scalar1=_CLAMP_MIN,
                scalar2=_CLAMP_MAX,
                op0=mybir.AluOpType.max,
                op1=mybir.AluOpType.min,
            )

            # Ship the clipped per-sample distances (4 KB); the host folds
            # them into the per-shard partial sums.
            nc.sync.dma_start(out=out_d.ap()[:, :], in_=dist[:])
    nc.compile()
    return nc


def _pack_labels(labels_shard: np.ndarray) -> np.ndarray:
    """idx[p, c] = labels[c*128 + p], int32 — matches the (p, c) layout."""
    return np.ascontiguousarray(labels_shard.reshape(_M, _P).T.astype(np.int32))


def _run(x, labels, centers, trace=False, **hw_kwargs):
    import ml_dtypes
    from concourse import bass_utils

    if "nc" not in _cache:
        _cache["nc"] = _build()
    nc = _cache["nc"]

    x = np.asarray(x)
    labels = np.ascontiguousarray(np.asarray(labels).astype(np.int64))
    centers = np.asarray(centers)
    assert x.shape == (_B, _D) and labels.shape == (_B,) and centers.shape == (_C, _D)
    assert labels.min() >= 0 and labels.max() < _C

    x_bf = x.astype(ml_dtypes.bfloat16)
    # bf16 pairs viewed as fp32 (see _build)
    cen_f32v = np.ascontiguousarray(centers.astype(ml_dtypes.bfloat16)).view(
        np.float32
    )

    in_maps = []
    for c in range(_N_CORES):
        sl = slice(c * _B_LOC, (c + 1) * _B_LOC)
        # x_perm[p, c*D:(c+1)*D] = x_shard[c*128 + p]
        x_perm = np.ascontiguousarray(
            x_bf[sl].reshape(_M, _P, _D).transpose(1, 0, 2).reshape(_P, _M * _D)
        )
        in_maps.append(
            {
                "x_perm": x_perm,
                "labels_packed": _pack_labels(labels[sl]),
                "centers_f32v": cen_f32v,
            }
        )

    r = bass_utils.run_bass_kernel_spmd(
        nc, in_maps, core_ids=list(range(_N_CORES)), trace=trace, **hw_kwargs
    )
    total = sum(res["out"].astype(np.float64).sum() for res in r.results)
    return np.array(total / _B, dtype=np.float32), r


def kernel(x, labels, centers):
    out, _ = _run(x, labels, centers, trace=False)
    return out


# revision 11
# speedup vs baseline: 1.3809x; 1.0966x over previous
"""CenterLoss kernel for Trainium2 (8 NeuronCores, data-parallel over batch).

loss = mean_i( clip( ||x_i - centers[labels[i]]||^2, 1e-12, 1e12 ) )

Instead of materializing the [B, C] distance matrix (as the reference does),
we gather the labeled center row per sample with indirect DMA and compute the
squared distance directly: O(B*D) work instead of O(B*C*D).

Sharding: x/labels split into 8 batch shards of 1024 rows; centers replicated.
Each core emits its shard's clipped per-sample distances; the host sums the
8 partials and divides by the global batch (the sanctioned all-reduce).

v4 design notes (trace-driven; this runtime's measured costs):
  - SWDGE descriptor-gen runs at ~9 ns/descriptor regardless of ucode
    (1024 gather descriptors ~ 9.2 us of Q7 time, serialized). The MoE
    dma_gather ucode batches calls but needs a library reload that costs
    ~12.6 us on this runtime, so mainline indirect_dma_start (no reload)
    with 8 gathers of 128 rows wins.
  - x and centers ship as bf16 (host-side cast): halves HBM traffic and
    speeds DVE. The Q7 indirect-DMA ucode miscomputes addresses for 2-byte
    dtypes, so centers are DECLARED fp32 [8000, 256] (bf16 pairs) and the
    gathered tile is bitcast back to bf16 for compute; address math then
    matches the working fp32 path exactly (1 KB rows).
  - idx loads via gpsimd SWDGE: the Q7 engine clears the framework preamble
    earliest (~6.1 us) and its queue is otherwise idle until the gathers;
    the scalar (Activation) HWDGE ring has ~4 us launch latency - avoid.
  - x loads in 2 halves on sync's HWDGE ring so the first 4 chunks can
    start compute ~2 us before the full 1 MB lands.
  - squared-row-sums: ScalarE takes 6 chunks (fused Square + fp32
    accumulator), DVE takes 2 (mult + reduce) plus all 8 bf16 subtracts.
  - loss error vs fp32 reference ~1e-3 << the 2e-2 gate (bf16 inputs,
    fp32 accumulation).

Per-core layout (B_loc=1024, P=128 partitions, M=8 chunks): sample s sits at
(partition p, chunk c) = (s%128, s//128); x is host-permuted to match, and
idx[p, c] = labels[c*128 + p].
"""

import sys

import numpy as np

if "/opt/trn_rl_repo" not in sys.path:
    sys.path.insert(0, "/opt/trn_rl_repo")

_B, _D, _C = 8192, 512, 8000
_N_CORES = 8
_B_LOC = _B // _N_CORES  # 1024 rows per core
_P = 128
_M = _B_LOC // _P  # 8 chunks of 128 rows
_DH = _D // 2  # fp32-typed width of a bf16 row (256)
_CLAMP_MIN, _CLAMP_MAX = 1e-12, 1e12

# engine assignment for the per-chunk squared row-sum. The LAST chunk goes to
# ScalarE (sub + fused square-accum is the shortest post-gather chain); DVE
# takes two mid chunks to keep ScalarE's queue from becoming the tail.
_DVE_SQ = (3, 6)

_cache: dict = {}


def _build():
    import concourse.bass as bass
    import concourse.tile as tile
    from concourse import bacc, mybir

    nc = bacc.Bacc(
        "TRN2",
        debug=False,
        enable_asserts=False,
        target_bir_lowering=False,
        num_devices=_N_CORES,
    )
    # x ships host-permuted: x_perm[p, c*D:(c+1)*D] = x_shard[c*128 + p] (bf16)
    x_d = nc.dram_tensor("x_perm", [_P, _M * _D], mybir.dt.bfloat16, kind="ExternalInput")
    # labels host-packed: idx[p, c] = labels[c*128 + p]
    lab_d = nc.dram_tensor("labels_packed", [_P, _M], mybir.dt.int32, kind="ExternalInput")
    # centers are bf16 pairs DECLARED fp32 so the gather's address math uses
    # the proven 4-byte path (1 KB per row either way).
    cen_d = nc.dram_tensor("centers_f32v", [_C, _DH], mybir.dt.float32, kind="ExternalInput")
    out_d = nc.dram_tensor("out", [_P, _M], mybir.dt.float32, kind="ExternalOutput")

    with tile.TileContext(nc) as tc:
        with (
            tc.tile_pool(name="big", bufs=1) as big,
            tc.tile_pool(name="work", bufs=4) as work,
            tc.tile_pool(name="misc", bufs=1) as misc,
        ):
            idx = misc.tile([_P, _M], mybir.dt.int32)
            # idx gates all gather descriptor-gen. sync's HWDGE ring has the
            # lowest push->semaphore latency on this runtime (~2.5 us vs ~3.8
            # for Q7 SWDGE and ~4.2+ for the Activation ring); push it first.
            nc.sync.dma_start(out=idx[:], in_=lab_d.ap())

            # x in 2 halves so early chunks unblock before the full MB lands
            xsb = big.tile([_P, _M * _D], mybir.dt.bfloat16)
            half_el = (_M // 2) * _D
            nc.sync.dma_start(
                out=xsb[:, :half_el], in_=x_d.ap()[:, :half_el]
            )
            nc.sync.dma_start(
                out=xsb[:, half_el:], in_=x_d.ap()[:, half_el:]
            )

            dist = misc.tile([_P, _M], mybir.dt.float32)

            g = big.tile([_P, _M * _DH], mybir.dt.float32)
            g3 = g[:].rearrange("p (m d) -> p m d", d=_DH)
            gb = g[:].bitcast(mybir.dt.bfloat16)  # [128, M*512] bf16 view
            for m in range(_M):
                nc.gpsimd.indirect_dma_start(
                    out=g3[:, m, :],
                    out_offset=None,
                    in_=cen_d.ap(),
                    in_offset=bass.IndirectOffsetOnAxis(
                        ap=idx[:, m : m + 1], axis=0
                    ),
                )
                diff = work.tile([_P, _D], mybir.dt.bfloat16, tag="diff")
                nc.vector.tensor_tensor(
                    out=diff[:],
                    in0=xsb[:, m * _D : (m + 1) * _D],
                    in1=gb[:, m * _D : (m + 1) * _D],
                    op=mybir.AluOpType.subtract,
                )
                if m not in _DVE_SQ:
                    sq = work.tile([_P, _D], mybir.dt.bfloat16, tag="sq")
                    nc.scalar.activation(
                        out=sq[:],
                        in_=diff[:],
                        func=mybir.ActivationFunctionType.Square,
                        accum_out=dist[:, m : m + 1],
                    )
                else:
                    sq = work.tile([_P, _D], mybir.dt.bfloat16, tag="sqv")
                    nc.vector.tensor_tensor(
                        out=sq[:], in0=diff[:], in1=diff[:],
                        op=mybir.AluOpType.mult,
                    )
                    nc.vector.tensor_reduce(
                        out=dist[:, m : m + 1],
                        in_=sq[:],
                        axis=mybir.AxisListType.X,
                        op=mybir.AluOpType.add,
                    )

            # clip both bounds: out = min(max(dist, lo), hi). Columns 0-6
            # clip as soon as they are done; only column 7's tiny clip trails
            # the final chunk, so the out-DMA fires sooner.
            nc.vector.tensor_scalar(
                out=dist[:, : _M - 1],
                in0=dist[:, : _M - 1],
                scalar1=_CLAMP_MIN,
                scalar2=_CLAMP_MAX,
                op0=mybir.AluOpType.max,
                op1=mybir.AluOpType.min,
            )
            nc.vector.tensor_scalar(
                out=dist[:, _M - 1 :],
                in0=dist[:, _M - 1 :],
                scalar1=_CLAMP_MIN,
                scalar2=_CLAMP_MAX,
                op0=mybir.AluOpType.max,
                op1=mybir.AluOpType.min,
            )

            # Ship the clipped per-sample distances (4 KB); the host folds
            # them into the per-shard partial sums.
            nc.sync.dma_start(out=out_d.ap()[:, :], in_=dist[:])
    nc.compile()
    return nc


def _pack_labels(labels_shard: np.ndarray) -> np.ndarray:
    """idx[p, c] = labels[c*128 + p], int32 — matches the (p, c) layout."""
    return np.ascontiguousarray(labels_shard.reshape(_M, _P).T.astype(np.int32))


def _run(x, labels, centers, trace=False, **hw_kwargs):
    import ml_dtypes
    from concourse import bass_utils

    if "nc" not in _cache:
        _cache["nc"] = _build()
    nc = _cache["nc"]

    x = np.asarray(x)
    labels = np.ascontiguousarray(np.asarray(labels).astype(np.int64))
    centers = np.asarray(centers)
    assert x.shape == (_B, _D) and labels.shape == (_B,) and centers.shape == (_C, _D)
    assert labels.min() >= 0 and labels.max() < _C

    x_bf = x.astype(ml_dtypes.bfloat16)
    # bf16 pairs viewed as fp32 (see _build)
    cen_f32v = np.ascontiguousarray(centers.astype(ml_dtypes.bfloat16)).view(
        np.float32
    )

    in_maps = []
    for c in range(_N_CORES):
        sl = slice(c * _B_LOC, (c + 1) * _B_LOC)
        # x_perm[p, c*D:(c+1)*D] = x_shard[c*128 + p]
        x_perm = np.ascontiguousarray(
            x_bf[sl].reshape(_M, _P, _D).transpose(1, 0, 2).reshape(_P, _M * _D)
        )
        in_maps.append(
            {
                "x_perm": x_perm,
                "labels_packed": _pack_labels(labels[sl]),
                "centers_f32v": cen_f32v,
            }
        )

    r = bass_utils.run_bass_kernel_spmd(
        nc, in_maps, core_ids=list(range(_N_CORES)), trace=trace, **hw_kwargs
    )
    total = sum(res["out"].astype(np.float64).sum() for res in r.results)
    return np.array(total / _B, dtype=np.float32), r


def kernel(x, labels, centers):
    out, _ = _run(x, labels, centers, trace=False)
    return out


# revision 13
# speedup vs baseline: 1.4162x; 1.0255x over previous
"""CenterLoss kernel for Trainium2 (8 NeuronCores, data-parallel over batch).

loss = mean_i( clip( ||x_i - centers[labels[i]]||^2, 1e-12, 1e12 ) )

Instead of materializing the [B, C] distance matrix (as the reference does),
we gather the labeled center row per sample with indirect DMA and compute the
squared distance directly: O(B*D) work instead of O(B*C*D).

Sharding: x/labels split into 8 batch shards of 1024 rows; centers replicated.
Each core emits its shard's clipped per-sample distances; the host sums the
8 partials and divides by the global batch (the sanctioned all-reduce).

v4 design notes (trace-driven; this runtime's measured costs):
  - SWDGE descriptor-gen runs at ~9 ns/descriptor regardless of ucode
    (1024 gather descriptors ~ 9.2 us of Q7 time, serialized). The MoE
    dma_gather ucode batches calls but needs a library reload that costs
    ~12.6 us on this runtime, so mainline indirect_dma_start (no reload)
    with 8 gathers of 128 rows wins.
  - x and centers ship as bf16 (host-side cast): halves HBM traffic and
    speeds DVE. The Q7 indirect-DMA ucode miscomputes addresses for 2-byte
    dtypes, so centers are DECLARED fp32 [8000, 256] (bf16 pairs) and the
    gathered tile is bitcast back to bf16 for compute; address math then
    matches the working fp32 path exactly (1 KB rows).
  - idx loads via gpsimd SWDGE: the Q7 engine clears the framework preamble
    earliest (~6.1 us) and its queue is otherwise idle until the gathers;
    the scalar (Activation) HWDGE ring has ~4 us launch latency - avoid.
  - x loads in 2 halves on sync's HWDGE ring so the first 4 chunks can
    start compute ~2 us before the full 1 MB lands.
  - squared-row-sums: ScalarE takes 6 chunks (fused Square + fp32
    accumulator), DVE takes 2 (mult + reduce) plus all 8 bf16 subtracts.
  - loss error vs fp32 reference ~1e-3 << the 2e-2 gate (bf16 inputs,
    fp32 accumulation).

Per-core layout (B_loc=1024, P=128 partitions, M=8 chunks): sample s sits at
(partition p, chunk c) = (s%128, s//128); x is host-permuted to match, and
idx[p, c] = labels[c*128 + p].
"""

import sys

import numpy as np

if "/opt/trn_rl_repo" not in sys.path:
    sys.path.insert(0, "/opt/trn_rl_repo")

_B, _D, _C = 8192, 512, 8000
_N_CORES = 8
_B_LOC = _B // _N_CORES  # 1024 rows per core
_P = 128
_M = _B_LOC // _P  # 8 chunks of 128 rows
_DH = _D // 2  # fp32-typed width of a bf16 row (256)
_CLAMP_MIN, _CLAMP_MAX = 1e-12, 1e12

# engine assignment for the per-chunk squared row-sum. The LAST chunk goes to
# ScalarE (sub + fused square-accum is the shortest post-gather chain); DVE
# takes two mid chunks to keep ScalarE's queue from becoming the tail.
_DVE_SQ = (3, 6)

_cache: dict = {}


def _build():
    import concourse.bass as bass
    import concourse.tile as tile
    from concourse import bacc, mybir

    nc = bacc.Bacc(
        "TRN2",
        debug=False,
        enable_asserts=False,
        target_bir_lowering=False,
        num_devices=_N_CORES,
        num_swdge_queues=4,
    )
    # x ships host-permuted: x_perm[p, c*D:(c+1)*D] = x_shard[c*128 + p] (bf16)
    x_d = nc.dram_tensor("x_perm", [_P, _M * _D], mybir.dt.bfloat16, kind="ExternalInput")
    # labels host-packed: idx[p, c] = labels[c*128 + p]
    lab_d = nc.dram_tensor("labels_packed", [_P, _M], mybir.dt.int32, kind="ExternalInput")
    # centers are bf16 pairs DECLARED fp32 so the gather's address math uses
    # the proven 4-byte path (1 KB per row either way).
    cen_d = nc.dram_tensor("centers_f32v", [_C, _DH], mybir.dt.float32, kind="ExternalInput")
    out_d = nc.dram_tensor("out", [_P, _M], mybir.dt.float32, kind="ExternalOutput")

    with tile.TileContext(nc) as tc:
        with (
            tc.tile_pool(name="big", bufs=1) as big,
            tc.tile_pool(name="work", bufs=4) as work,
            tc.tile_pool(name="misc", bufs=1) as misc,
        ):
            idx = misc.tile([_P, _M], mybir.dt.int32)
            # idx gates all gather descriptor-gen. sync's HWDGE ring has the
            # lowest push->semaphore latency on this runtime (~2.5 us vs ~3.8
            # for Q7 SWDGE and ~4.2+ for the Activation ring); push it first.
            nc.sync.dma_start(out=idx[:], in_=lab_d.ap())

            # x in 2 halves so early chunks unblock before the full MB lands
            xsb = big.tile([_P, _M * _D], mybir.dt.bfloat16)
            half_el = (_M // 2) * _D
            nc.sync.dma_start(
                out=xsb[:, :half_el], in_=x_d.ap()[:, :half_el]
            )
            nc.sync.dma_start(
                out=xsb[:, half_el:], in_=x_d.ap()[:, half_el:]
            )

            dist = misc.tile([_P, _M], mybir.dt.float32)

            g = big.tile([_P, _M * _DH], mybir.dt.float32)
            g3 = g[:].rearrange("p (m d) -> p m d", d=_DH)
            gb = g[:].bitcast(mybir.dt.bfloat16)  # [128, M*512] bf16 view
            for m in range(_M):
                ginst = nc.gpsimd.indirect_dma_start(
                    out=g3[:, m, :],
                    out_offset=None,
                    in_=cen_d.ap(),
                    in_offset=bass.IndirectOffsetOnAxis(
                        ap=idx[:, m : m + 1], axis=0
                    ),
                )
                # spread gathers over the 4 SWDGE queues: the 16 KB descriptor
                # carveout only holds ~4 gathers' descriptors, so a single
                # queue stalls on ring reclaim from gather 5 onward.
                qn = m % 4
                ginst.ins.queue = f"qPoolDynamic{qn}" if qn else "qPoolDynamic"
                diff = work.tile([_P, _D], mybir.dt.bfloat16, tag="diff")
                nc.vector.tensor_tensor(
                    out=diff[:],
                    in0=xsb[:, m * _D : (m + 1) * _D],
                    in1=gb[:, m * _D : (m + 1) * _D],
                    op=mybir.AluOpType.subtract,
                )
                if m not in _DVE_SQ:
                    sq = work.tile([_P, _D], mybir.dt.bfloat16, tag="sq")
                    nc.scalar.activation(
                        out=sq[:],
                        in_=diff[:],
                        func=mybir.ActivationFunctionType.Square,
                        accum_out=dist[:, m : m + 1],
                    )
                else:
                    sq = work.tile([_P, _D], mybir.dt.bfloat16, tag="sqv")
                    nc.vector.tensor_tensor(
                        out=sq[:], in0=diff[:], in1=diff[:],
                        op=mybir.AluOpType.mult,
                    )
                    nc.vector.tensor_reduce(
                        out=dist[:, m : m + 1],
                        in_=sq[:],
                        axis=mybir.AxisListType.X,
                        op=mybir.AluOpType.add,
                    )

            # clip both bounds: out = min(max(dist, lo), hi). Columns 0-6
            # clip as soon as they are done; only column 7's tiny clip trails
            # the final chunk, so the out-DMA fires sooner.
            nc.vector.tensor_scalar(
                out=dist[:, : _M - 1],
                in0=dist[:, : _M - 1],
                scalar1=_CLAMP_MIN,
                scalar2=_CLAMP_MAX,
                op0=mybir.AluOpType.max,
                op1=mybir.AluOpType.min,
            )
            nc.vector.tensor_scalar(
                out=dist[:, _M - 1 :],
                in0=dist[:, _M - 1 :],
                scalar1=_CLAMP_MIN,
                scalar2=_CLAMP_MAX,
                op0=mybir.AluOpType.max,
                op1=mybir.AluOpType.min,
            )

            # Ship the clipped per-sample distances (4 KB); the host folds
            # them into the per-shard partial sums.
            nc.sync.dma_start(out=out_d.ap()[:, :], in_=dist[:])
    nc.compile()
    return nc


def _pack_labels(labels_shard: np.ndarray) -> np.ndarray:
    """idx[p, c] = labels[c*128 + p], int32 — matches the (p, c) layout."""
    return np.ascontiguousarray(labels_shard.reshape(_M, _P).T.astype(np.int32))


def _run(x, labels, centers, trace=False, **hw_kwargs):
    import ml_dtypes
    from concourse import bass_utils

    if "nc" not in _cache:
        _cache["nc"] = _build()
    nc = _cache["nc"]

    x = np.asarray(x)
    labels = np.ascontiguousarray(np.asarray(labels).astype(np.int64))
    centers = np.asarray(centers)
    assert x.shape == (_B, _D) and labels.shape == (_B,) and centers.shape == (_C, _D)
    assert labels.min() >= 0 and labels.max() < _C

    x_bf = x.astype(ml_dtypes.bfloat16)
    # bf16 pairs viewed as fp32 (see _build)
    cen_f32v = np.ascontiguousarray(centers.astype(ml_dtypes.bfloat16)).view(
        np.float32
    )

    in_maps = []
    for c in range(_N_CORES):
        sl = slice(c * _B_LOC, (c + 1) * _B_LOC)
        # x_perm[p, c*D:(c+1)*D] = x_shard[c*128 + p]
        x_perm = np.ascontiguousarray(
            x_bf[sl].reshape(_M, _P, _D).transpose(1, 0, 2).reshape(_P, _M * _D)
        )
        in_maps.append(
            {
                "x_perm": x_perm,
                "labels_packed": _pack_labels(labels[sl]),
                "centers_f32v": cen_f32v,
            }
        )

    r = bass_utils.run_bass_kernel_spmd(
        nc, in_maps, core_ids=list(range(_N_CORES)), trace=trace, **hw_kwargs
    )
    total = sum(res["out"].astype(np.float64).sum() for res in r.results)
    return np.array(total / _B, dtype=np.float32), r


def kernel(x, labels, centers):
    out, _ = _run(x, labels, centers, trace=False)
    return out
